# revision 1
# baseline (speedup 1.0000x reference)
"""CACombiner Trainium2 kernel: conv-projected efficient attention + FFN.

Data-parallel over batch: 8 batch elements -> 8 NeuronCores, identical SPMD
program per core. All heavy matmuls run as float32r (full PE rate); the
attention-weight path (exp(k), v, softmax(q), ctx) runs in bf16.
"""
import sys
sys.path.insert(0, "/opt/trn_rl_repo")
from contextlib import ExitStack

import numpy as np

import concourse.bass as bass
import concourse.tile as tile
from concourse import mybir, bacc
from concourse.bass_utils import run_bass_kernel_spmd
from concourse.alu_op_type import AluOpType

F32 = mybir.dt.float32
F32R = mybir.dt.float32r
BF16 = mybir.dt.bfloat16
AFT = mybir.ActivationFunctionType
Ax = mybir.AxisListType

B, C, L = 8, 512, 4096
H, DK = 8, 64
EPS = 1e-5
CC = C // 128          # 4 channel chunks
NL1 = L // 128         # 32 phase-1 l-tiles
NL2 = L // 512         # 8 phase-2 l-tiles

_CACHE = {}
LAST_RESULT = None


def _build_program():
    nc = bacc.Bacc("TRN2", target_bir_lowering=False, debug=False)

    def din(name, shape, dtype):
        return nc.dram_tensor(name, list(shape), dtype, kind="ExternalInput").ap()

    z1d = din("z1", (C, L), F32R)
    z2d = din("z2", (C, L), F32R)
    WqTt_d = din("WqTt", (128, CC, 512), F32R)
    bq_row_d = din("bq_row", (1, 512), F32R)
    WkvTt_d = din("WkvTt", (128, CC, 1024), F32R)
    WrTt_d = din("WrTt", (128, CC, 512), F32R)
    W1gTt_d = din("W1gTt", (128, CC, 1024), F32R)
    W2gTt_d = din("W2gTt", (128, 8, 512), F32R)
    U1W_d = din("U1W", (2, 1024), F32R)
    u2ct_d = din("u2ct", (128, 8), F32R)
    G2B_d = din("G2B", (2, 512), F32R)
    ivgt_d = din("ivgt", (128, CC), F32R)
    inv512_d = din("inv512", (128, 1), F32R)
    ones1x128_d = din("ones1x128", (1, 128), F32R)
    ident_d = din("ident", (128, 128), BF16)
    br_c_d = din("br_c", (128, CC), F32)
    bv_c_d = din("bv_c", (128, CC), F32)
    be2_c_d = din("be2_c", (128, CC), F32)
    eps_c_d = din("eps_c", (128, 1), F32)
    ones_row_d = din("ones_row", (1, 512), F32R)
    outd = nc.dram_tensor("out", [C, L], F32, kind="ExternalOutput").ap()

    z1r = z1d.rearrange("(cc p) l -> p cc l", p=128)
    z2r = z2d.rearrange("(cc p) l -> p cc l", p=128)

    mm = nc.tensor.matmul
    tt = nc.vector.tensor_tensor
    ts = nc.vector.tensor_scalar
    stt = nc.vector.scalar_tensor_tensor
    act = nc.scalar.activation

    with tile.TileContext(nc) as tc, ExitStack() as ctx:
        cpool = ctx.enter_context(tc.tile_pool(name="consts", bufs=1))

        def const_tile(shape, dtype, src, tag):
            t = cpool.tile(list(shape), dtype, tag=tag, name=tag)
            nc.sync.dma_start(t[:], src)
            return t

        WqTt = const_tile((128, CC, 512), F32R, WqTt_d, "WqTt")
        bq_row = const_tile((1, 512), F32R, bq_row_d, "bq_row")
        WkvTt = const_tile((128, CC, 1024), F32R, WkvTt_d, "WkvTt")
        WrTt = const_tile((128, CC, 512), F32R, WrTt_d, "WrTt")
        W1gTt = const_tile((128, CC, 1024), F32R, W1gTt_d, "W1gTt")
        W2gTt = const_tile((128, 8, 512), F32R, W2gTt_d, "W2gTt")
        U1W = const_tile((2, 1024), F32R, U1W_d, "U1W")
        u2ct = const_tile((128, 8), F32R, u2ct_d, "u2ct")
        G2B = const_tile((2, 512), F32R, G2B_d, "G2B")
        ivgt = const_tile((128, CC), F32R, ivgt_d, "ivgt")
        inv512 = const_tile((128, 1), F32R, inv512_d, "inv512")
        ones1x128 = const_tile((1, 128), F32R, ones1x128_d, "ones1x128")
        ident = const_tile((128, 128), BF16, ident_d, "ident")
        br_c = const_tile((128, CC), F32, br_c_d, "br_c")
        bv_c = const_tile((128, CC), F32, bv_c_d, "bv_c")
        be2_c = const_tile((128, CC), F32, be2_c_d, "be2_c")
        eps_c = const_tile((128, 1), F32, eps_c_d, "eps_c")
        ones_row = const_tile((1, 512), F32R, ones_row_d, "ones_row")

        # persistent across phases
        qsm = cpool.tile([128, CC, L], BF16, tag="qsm", name="qsm")      # softmaxed q, channels-first
        ctxbd = [cpool.tile([128, 128], BF16, tag=f"ctxbd{p}", name=f"ctxbd{p}") for p in range(CC)]

        # ---------------- Phase 1: q softmax + k/v + ctx accumulation ----------------
        with ExitStack() as p1:
            lp1 = p1.enter_context(tc.tile_pool(name="lp1", bufs=2))
            ps_ctx = p1.enter_context(tc.tile_pool(name="ps_ctx", bufs=1, space="PSUM"))
            ps_w = p1.enter_context(tc.tile_pool(name="ps_w", bufs=1, space="PSUM"))

            ctxps = [ps_ctx.tile([128, 129], F32, tag=f"ctx{p}", name=f"ctxps{p}") for p in range(CC)]

            for lt in range(NL1):
                sl = slice(lt * 128, (lt + 1) * 128)
                z1c = lp1.tile([128, CC, 128], F32R, tag="z1c")
                nc.sync.dma_start(z1c[:], z1r[:, :, sl])
                z2c = lp1.tile([128, CC, 128], F32R, tag="z2c")
                nc.sync.dma_start(z2c[:], z2r[:, :, sl])

                # qT [l,128][o,512] = z1^T Wq^T + bq
                qps = ps_w.tile([128, 512], F32, tag="qps")
                for cc in range(CC):
                    mm(qps[:], z1c[:, cc, :], WqTt[:, cc, :], start=(cc == 0), stop=False)
                mm(qps[:], ones1x128[:], bq_row[:], start=False, stop=True)

                # exp + per-head sums (ACT accumulate), then normalize
                EqT = lp1.tile([128, 512], F32, tag="EqT")
                Sq = lp1.tile([128, 8], F32, tag="Sq")
                for h in range(H):
                    hs = slice(h * 64, (h + 1) * 64)
                    act(EqT[:, hs], qps[:, hs], AFT.Exp, accum_out=Sq[:, h:h + 1])
                rq = lp1.tile([128, 8], F32, tag="rq")
                nc.vector.reciprocal(rq[:], Sq[:])
                qsmT = lp1.tile([128, 512], BF16, tag="qsmT")
                tt(qsmT[:].rearrange("p (g x) -> p g x", x=64),
                   EqT[:].rearrange("p (g x) -> p g x", x=64),
                   rq[:].unsqueeze(2).broadcast_to([128, 8, 64]), AluOpType.mult)

                # transpose qsmT back to channels-first into qsm
                tps = ps_w.tile([128, 512], BF16, tag="tps")
                for cc in range(CC):
                    cs = slice(cc * 128, (cc + 1) * 128)
                    nc.tensor.transpose(tps[:, cs], qsmT[:, cs], ident[:])
                nc.vector.tensor_copy(
                    qsm[:, :, sl],
                    tps[:].rearrange("p (cc x) -> p cc x", x=128))

                # kT | vT
                kvps = ps_w.tile([128, 1024], F32, tag="kvps")
                for cc in range(CC):
                    mm(kvps[:, 0:512], z2c[:, cc, :], WkvTt[:, cc, 0:512],
                       start=(cc == 0), stop=(cc == CC - 1))
                for cc in range(CC):
                    mm(kvps[:, 512:1024], z2c[:, cc, :], WkvTt[:, cc, 512:1024],
                       start=(cc == 0), stop=(cc == CC - 1))
                EkT = lp1.tile([128, 512], BF16, tag="EkT")
                act(EkT[:], kvps[:, 0:512], AFT.Exp)
                vT = lp1.tile([128, 516], BF16, tag="vT")
                nc.vector.tensor_copy(
                    vT[:].rearrange("p (pr x) -> p pr x", pr=4)[:, :, 0:128],
                    kvps[:, 512:1024].rearrange("p (pr x) -> p pr x", pr=4))
                nc.vector.memset(vT[:].rearrange("p (pr x) -> p pr x", pr=4)[:, :, 128:129], 1.0)

                # ctx accumulation: per head-pair [2heads-k, 2heads-v | S]
                for pr in range(CC):
                    mm(ctxps[pr][:], EkT[:, pr * 128:(pr + 1) * 128],
                       vT[:, pr * 129:(pr + 1) * 129],
                       start=(lt == 0), stop=(lt == NL1 - 1), skip_group_check=True)

            # finalize ctx: normalize rows by S, build block-diagonal bf16 tiles
            for pr in range(CC):
                rs = lp1.tile([128, 1], F32, tag="rs")
                nc.vector.reciprocal(rs[:], ctxps[pr][:, 128:129])
                nc.vector.memset(ctxbd[pr][:], 0.0)
                ts(ctxbd[pr][0:64, 0:64], ctxps[pr][0:64, 0:64], rs[0:64, :], None,
                   AluOpType.mult)
                ts(ctxbd[pr][64:128, 64:128], ctxps[pr][64:128, 64:128], rs[64:128, :], None,
                   AluOpType.mult)

        # ---------------- Phase 2: attention apply + reprojection + LN/FFN ----------------
        with ExitStack() as p2:
            lp2 = p2.enter_context(tc.tile_pool(name="lp2", bufs=2))
            lph = p2.enter_context(tc.tile_pool(name="lph", bufs=1))
            ps_big = p2.enter_context(tc.tile_pool(name="ps_big", bufs=5, space="PSUM"))
            ps_row = p2.enter_context(tc.tile_pool(name="ps_row", bufs=2, space="PSUM"))

            for lt in range(NL2):
                sl = slice(lt * 512, (lt + 1) * 512)
                z1res = lp2.tile([128, CC, 512], F32R, tag="z1res", bufs=1)
                nc.sync.dma_start(z1res[:], z1r[:, :, sl])

                # att[v,l] = ctx_bd @ qsm + bv
                att = []
                for pr in range(CC):
                    aps = ps_big.tile([128, 512], F32, tag="big")
                    mm(aps[:], ctxbd[pr][:], qsm[:, pr, sl], start=True, stop=True)
                    a = lph.tile([128, 512], F32R, tag=f"att{pr}")
                    ts(a[:], aps[:], bv_c[:, pr:pr + 1], None, AluOpType.add)
                    att.append(a)

                # z = Wr att + br + z1
                zt = []
                for ot in range(CC):
                    zps = ps_big.tile([128, 512], F32, tag="big")
                    for pr in range(CC):
                        mm(zps[:], WrTt[:, pr, ot * 128:(ot + 1) * 128], att[pr][:],
                           start=(pr == 0), stop=(pr == CC - 1))
                    z = lph.tile([128, 512], F32R, tag=f"z{ot}")
                    stt(z[:], zps[:], br_c[:, ot:ot + 1], z1res[:, ot, :].bitcast(F32),
                        AluOpType.add, AluOpType.add)
                    zt.append(z)

                # LN1 stats rows
                mups = ps_row.tile([1, 512], F32, tag="row")
                for ot in range(CC):
                    mm(mups[:], inv512[:], zt[ot][:], start=(ot == 0), stop=(ot == CC - 1))
                e2ps = ps_row.tile([1, 512], F32, tag="row")
                for ot in range(CC):
                    zsq = lp2.tile([128, 512], F32R, tag="zsq")
                    act(zsq[:], zt[ot][:].bitcast(F32), AFT.Square)
                    mm(e2ps[:], inv512[:], zsq[:], start=(ot == 0), stop=(ot == CC - 1))
                murow = lp2.tile([1, 512], F32, tag="murow", bufs=1)
                nc.vector.tensor_copy(murow[:], mups[:])
                musq = lp2.tile([1, 512], F32, tag="musq", bufs=1)
                tt(musq[:], murow[:], murow[:], AluOpType.mult)
                varrow = lp2.tile([1, 512], F32, tag="varrow", bufs=1)
                tt(varrow[:], e2ps[:], musq[:], AluOpType.subtract)
                sig = lp2.tile([1, 512], F32, tag="sig", bufs=1)
                act(sig[:], varrow[:], AFT.Sqrt, bias=eps_c[0:1, :])
                rhs2 = lp2.tile([2, 512], F32R, tag="rhs2", bufs=1)
                ts(rhs2[0:1, :], mups[:], -1.0, None, AluOpType.mult)
                sigR = lp2.tile([1, 512], F32R, tag="sigR", bufs=1)
                nc.vector.tensor_copy(sigR[:], sig[:])
                nc.sync.dma_start(rhs2[1:2, :], sigR[:])
                invsF = lp2.tile([1, 512], F32, tag="invsF", bufs=1)
                nc.vector.reciprocal(invsF[:], sig[:])
                invs = lp2.tile([1, 512], F32R, tag="invs", bufs=1)
                nc.vector.tensor_copy(invs[:], invsF[:])
                bc = ps_big.tile([128, 512], F32, tag="big")
                mm(bc[:], ones1x128[:], invs[:], start=True, stop=True)
                invsb = lp2.tile([128, 512], F32, tag="invsb", bufs=1)
                nc.vector.tensor_copy(invsb[:], bc[:])

                # FFN1 + ELU + FFN2 accumulation (j-outer so hE slots rotate)
                f2ps = [ps_big.tile([128, 512], F32, tag="big", name=f"f2ps{o2}")
                        for o2 in range(CC)]
                mu2 = ps_row.tile([1, 512], F32, tag="row", name="mu2")
                for j in range(8):
                    fps = ps_big.tile([128, 512], F32, tag="big", name="fps")
                    for cc in range(CC):
                        mm(fps[:], W1gTt[:, cc, j * 128:(j + 1) * 128], zt[cc][:],
                           start=(cc == 0), stop=False)
                    mm(fps[:], U1W[:, j * 128:(j + 1) * 128], rhs2[:], start=False, stop=True)
                    hp = lp2.tile([128, 512], F32, tag="hp")
                    tt(hp[:], fps[:], invsb[:], AluOpType.mult)
                    E = lp2.tile([128, 512], F32, tag="E")
                    act(E[:], hp[:], AFT.Exp)
                    nc.gpsimd.tensor_scalar(E[:], E[:], 1.0, -1.0, AluOpType.min,
                                            AluOpType.add)
                    he = lph.tile([128, 512], F32R, tag="hE", bufs=3, name="he")
                    stt(he[:], hp[:], 0.0, E[:], AluOpType.max, AluOpType.add)
                    for o2 in range(CC):
                        mm(f2ps[o2][:], W2gTt[:, j, o2 * 128:(o2 + 1) * 128], he[:],
                           start=(j == 0), stop=False, skip_group_check=True)
                    mm(mu2[:], u2ct[:, j:j + 1], he[:], start=(j == 0), stop=(j == 7),
                       skip_group_check=True)
                rhs2b = lp2.tile([2, 512], F32R, tag="rhs2b", bufs=1)
                nc.sync.dma_start(rhs2b[0:1, :], ones_row[:])
                negmu2 = lp2.tile([1, 512], F32R, tag="negmu2", bufs=1)
                ts(negmu2[:], mu2[:], -1.0, B2MEAN_PLACEHOLDER, AluOpType.mult,
                   AluOpType.subtract)
                nc.sync.dma_start(rhs2b[1:2, :], negmu2[:])
                yg = []
                for o2 in range(CC):
                    mm(f2ps[o2][:], G2B[:, o2 * 128:(o2 + 1) * 128], rhs2b[:],
                       start=False, stop=True, skip_group_check=True)
                    y = lph.tile([128, 512], F32, tag=f"yg{o2}", name=f"yg{o2}")
                    nc.vector.tensor_copy(y[:], f2ps[o2][:])
                    yg.append(y)

                # LN2 variance + apply
                v2ps = ps_row.tile([1, 512], F32, tag="row")
                for o2 in range(CC):
                    sq2 = lp2.tile([128, 512], F32R, tag="sq2")
                    act(sq2[:], yg[o2][:], AFT.Square)
                    mm(v2ps[:], ivgt[:, o2:o2 + 1], sq2[:], start=(o2 == 0),
                       stop=(o2 == CC - 1))
                sig2 = lp2.tile([1, 512], F32, tag="sig2", bufs=1)
                act(sig2[:], v2ps[:], AFT.Sqrt, bias=eps_c[0:1, :])
                invs2F = lp2.tile([1, 512], F32, tag="invs2F", bufs=1)
                nc.vector.reciprocal(invs2F[:], sig2[:])
                invs2 = lp2.tile([1, 512], F32R, tag="invs2", bufs=1)
                nc.vector.tensor_copy(invs2[:], invs2F[:])
                bc2 = ps_big.tile([128, 512], F32, tag="big")
                mm(bc2[:], ones1x128[:], invs2[:], start=True, stop=True)
                invsb2 = lp2.tile([128, 512], F32, tag="invsb2", bufs=1)
                nc.vector.tensor_copy(invsb2[:], bc2[:])
                for o2 in range(CC):
                    tt(yg[o2][:], yg[o2][:], invsb2[:], AluOpType.mult)
                    ot_t = lp2.tile([128, 512], F32, tag="ot")
                    nc.gpsimd.tensor_scalar(ot_t[:], yg[o2][:], be2_c[:, o2:o2 + 1],
                                            None, AluOpType.add)
                    nc.sync.dma_start(outd[o2 * 128:(o2 + 1) * 128, sl], ot_t[:])

    nc.compile()
    return nc


def _prep_consts(Wq, bq, Wk, bk, Wv, bv, Wr, br, g1, be1, W1, b1, W2, b2, g2, be2):
    f = np.float32
    WqT = np.ascontiguousarray(Wq.T, dtype=f)                       # [c, o]
    WkvT = np.concatenate([Wk.T, Wv.T], axis=1).astype(f)           # [c, k|v]
    WrT = np.ascontiguousarray(Wr.T, dtype=f)                       # [v, o]
    W1g = (W1 * g1[None, :]).astype(f)                              # [1024, c]
    W1gT = np.ascontiguousarray(W1g.T)                              # [c, 1024]
    W2g = (W2 * g2[:, None]).astype(f)                              # [c, 1024h]
    W2gT = np.ascontiguousarray(W2g.T)                              # [h, c]
    u1 = W1g.sum(axis=1).astype(f)
    w1bb = (W1 @ be1 + b1).astype(f)
    u2 = (W2.sum(axis=0) / 512.0).astype(f)
    ivg = (1.0 / (512.0 * g2 * g2)).astype(f)
    b2mean = float(np.mean(b2))

    def chunkT(a, n):          # [n*128, m] -> [128, n, m]
        return np.ascontiguousarray(a.reshape(n, 128, -1).transpose(1, 0, 2))

    def colsT(v, n):           # [n*128] -> [128, n]
        return np.ascontiguousarray(v.reshape(n, 128).T)

    return {
        "WqTt": chunkT(WqT, CC),
        "bq_row": bq.reshape(1, 512).astype(f),
        "WkvTt": chunkT(WkvT, CC),
        "WrTt": chunkT(WrT, CC),
        "W1gTt": chunkT(W1gT, CC),
        "W2gTt": chunkT(W2gT, 8),
        "U1W": np.stack([u1, w1bb]).astype(f),
        "u2ct": colsT(u2, 8),
        "G2B": np.stack([(g2 * b2).astype(f), g2.astype(f)]),
        "ivgt": colsT(ivg, CC),
        "inv512": np.full((128, 1), 1.0 / 512.0, dtype=f),
        "ones1x128": np.ones((1, 128), dtype=f),
        "ident": np.eye(128, dtype=f).astype(np.dtype("bfloat16") if False else f),
        "br_c": colsT(br.astype(f), CC),
        "bv_c": colsT(bv.astype(f), CC),
        "be2_c": colsT(be2.astype(f), CC),
        "eps_c": np.full((128, 1), EPS, dtype=f),
        "ones_row": np.ones((1, 512), dtype=f),
    }, b2mean


def kernel(**inputs):
    global LAST_RESULT
    import ml_dtypes
    z1 = np.asarray(inputs["z1"], dtype=np.float32)
    z2 = np.asarray(inputs["z2"], dtype=np.float32)
    consts, b2mean = _prep_consts(
        *[np.asarray(inputs[k], dtype=np.float32) for k in
          ["Wq", "bq", "Wk", "bk", "Wv", "bv", "Wr", "br", "g1", "be1",
           "W1", "b1", "W2", "b2", "g2", "be2"]])
    consts["ident"] = np.eye(128, dtype=ml_dtypes.bfloat16)

    key = ("prog", round(b2mean * 1e9))
    if key not in _CACHE:
        global B2MEAN_PLACEHOLDER
        B2MEAN_PLACEHOLDER = b2mean
        _CACHE.clear()
        _CACHE[key] = _build_program()
    nc = _CACHE[key]

    in_maps = []
    for b in range(B):
        m = dict(consts)
        m["z1"] = np.ascontiguousarray(z1[b])
        m["z2"] = np.ascontiguousarray(z2[b])
        in_maps.append(m)

    import os
    trace = bool(int(os.environ.get("KERNEL_TRACE", "0")))
    res = run_bass_kernel_spmd(nc, in_maps, list(range(B)), trace=trace)
    LAST_RESULT = res
    out = np.stack([res.results[b]["out"] for b in range(B)], axis=0)
    return out.astype(np.float32)


B2MEAN_PLACEHOLDER = 0.0



# revision 59
# speedup vs baseline: 2.9403x; 2.9403x over previous
"""CACombiner Trainium2 kernel: conv-projected efficient attention + FFN.

Data-parallel over batch: 8 batch elements -> 8 NeuronCores, identical SPMD
program per core. Attention path (q/k/v projections, ctx, and the fused
Wr@ctx reprojection) runs in fp8e4 with DoubleRow matmuls (2 k-tiles per
instruction, 0.5 cycles/row); the FFN runs in bf16. LayerNorms are fused:
LN1 mean/var via PE row-matmuls + gpsimd partition-broadcast, LN2 likewise
with g2/b2 folded on host.
"""
import sys
sys.path.insert(0, "/opt/trn_rl_repo")
from contextlib import ExitStack

import numpy as np

import concourse.bass as bass
import concourse.tile as tile
from concourse import mybir, bacc
from concourse.bass_utils import run_bass_kernel_spmd
from concourse.alu_op_type import AluOpType

F32 = mybir.dt.float32
F32R = mybir.dt.float32r
BF16 = mybir.dt.bfloat16
F8 = mybir.dt.float8e4
AFT = mybir.ActivationFunctionType
DR = mybir.MatmulPerfMode.DoubleRow

# Every activation this kernel uses (Exp, Ln, Relu, Copy, Square, Identity)
# lives together in one hardware activation-table set. The default chooser
# picks the first set containing each function, which alternates sets and
# inserts a 1.3us LoadActFuncSet per switch. Narrow the chooser's view so the
# all-inclusive set is the unique provider (names/indexes preserved, so the
# emitted act_func_set_id still refers to the true table).
_OUR_FUNCS = {AFT.Exp, AFT.Ln, AFT.Relu, AFT.Copy, AFT.Square, AFT.Identity}


def _patch_act_tables():
    import concourse.hw_specs as hw_specs
    import concourse.bacc as bacc_mod
    orig = hw_specs.get_activation_tables
    if getattr(hw_specs, "_cac_patched", False):
        return

    def patched(arch):
        t = orig(arch)
        keep = None
        for name, s in t.items():
            if _OUR_FUNCS <= s:
                keep = name
                break
        if keep is None:
            return t
        return {name: (s if name == keep else s - _OUR_FUNCS)
                for name, s in t.items()}

    hw_specs.get_activation_tables = patched
    bacc_mod.get_activation_tables = patched
    hw_specs._cac_patched = True

B, C, L = 8, 512, 4096
H, DK = 8, 64
EPS = 1e-5
CC = C // 128            # 4 channel chunks
NT = L // 512            # 8 outer l-tiles
SW = 32.0                # fp8 scale for Wq/Wk/Wv
SQ = 64.0                # fp8 scale for softmaxed q
SC = 256.0               # fp8 scale for W_comb = Wr @ ctx

_CACHE = {}
LAST_RESULT = None


def _build_program(flags):
    bq_nz, br_nz, b1_nz, b2_nz, be2_nz = flags
    _patch_act_tables()
    nc = bacc.Bacc("TRN2", target_bir_lowering=False, debug=False)

    def din(name, shape, dtype):
        return nc.dram_tensor(name, list(shape), dtype, kind="ExternalInput").ap()

    z1_8d = din("z1_8", (128, CC, L), F8)
    z2_8d = din("z2_8", (128, CC, L), F8)
    z1res_d = din("z1res", (128, CC, L), BF16)
    Wq8T_d = din("Wq8T", (128, CC, 512), F8)
    Wkv8T_d = din("Wkv8T", (128, CC, 1024), F8)
    hm8_d = din("hm8", (128, CC, 16), F8)
    hm64_d = din("hm64", (8, CC, 128), F32R)
    WrTb_d = din("WrTb", (128, CC, 512), BF16)
    W1gTb_d = din("W1gTb", (128, CC, 1024), BF16)
    W2gTb_d = din("W2gTb", (128, 8, 512), BF16)
    u2ct_d = din("u2ct", (128, 8), BF16)
    inv512_d = din("inv512", (128, 1), BF16)
    ivg8_d = din("ivg8", (128, CC, 16), F8)
    ones8p_d = din("ones8p", (128, CC, 16), F8)
    g2c_d = din("g2c", (128, CC), F32)
    identb_d = din("identb", (128, 128), BF16)
    eps_d = din("epsA", (1, 1), F32)
    ones_row_d = din("ones_row", (1, 512), F32R)
    bq_r_d = din("bq_r", (1, 512), F32R)
    br_c_d = din("br_c", (128, CC), F32)
    w1bb_r_d = din("w1bb_r", (1, 1024), F32R)
    g2b2_r_d = din("g2b2_r", (1, 512), F32R)
    be2_c_d = din("be2_c", (128, CC), F32)
    outd = nc.dram_tensor("out", [C, L], BF16, kind="ExternalOutput").ap()

    mm = nc.tensor.matmul
    tt = nc.vector.tensor_tensor
    ts = nc.vector.tensor_scalar
    stt = nc.vector.scalar_tensor_tensor
    act = nc.scalar.activation
    gp = nc.gpsimd

    with tile.TileContext(nc) as tc, ExitStack() as ctx:
        cpool = ctx.enter_context(tc.tile_pool(name="consts", bufs=1))

        def const_tile(shape, dtype, src, tag):
            t = cpool.tile(list(shape), dtype, tag=tag, name=tag)
            nc.sync.dma_start(t[:], src)
            return t

        # phase-1 weights first so the first q matmuls aren't queued behind
        # the big FFN weight transfers; the rest loads during phase 1
        Wq8T = const_tile((128, CC, 512), F8, Wq8T_d, "Wq8T")
        Wkv8T = const_tile((128, CC, 1024), F8, Wkv8T_d, "Wkv8T")
        hm8 = const_tile((128, CC, 16), F8, hm8_d, "hm8")
        hm64 = const_tile((8, CC, 128), F32R, hm64_d, "hm64")
        identb = const_tile((128, 128), BF16, identb_d, "identb")
        epsA = const_tile((1, 1), F32, eps_d, "epsA")
        ones_row = const_tile((1, 512), F32R, ones_row_d, "ones_row")
        if bq_nz:
            bq_r = const_tile((1, 512), F32R, bq_r_d, "bq_r")

        def load_late_consts():
            c = {}
            c["WrTb"] = const_tile((128, CC, 512), BF16, WrTb_d, "WrTb")
            c["W1gTb"] = const_tile((128, CC, 1024), BF16, W1gTb_d, "W1gTb")
            c["W2gTb"] = const_tile((128, 8, 512), BF16, W2gTb_d, "W2gTb")
            c["u2ct"] = const_tile((128, 8), BF16, u2ct_d, "u2ct")
            c["inv512"] = const_tile((128, 1), BF16, inv512_d, "inv512")
            c["ivg8"] = const_tile((128, CC, 16), F8, ivg8_d, "ivg8")
            c["ones8p"] = const_tile((128, CC, 16), F8, ones8p_d, "ones8p")
            c["g2c"] = const_tile((128, CC), F32, g2c_d, "g2c")
            if br_nz:
                c["br_c"] = const_tile((128, CC), F32, br_c_d, "br_c")
            if b1_nz:
                c["w1bb_r"] = const_tile((1, 1024), F32R, w1bb_r_d, "w1bb_r")
            if b2_nz:
                c["g2b2_r"] = const_tile((1, 512), F32R, g2b2_r_d, "g2b2_r")
            if be2_nz:
                c["be2_c"] = const_tile((128, CC), F32, be2_c_d, "be2_c")
            return c

        # persistent across phases
        qsm8 = cpool.tile([128, CC, L], F8, tag="qsm8", name="qsm8")
        WcT8 = cpool.tile([128, CC, 512], F8, tag="WcT8", name="WcT8")

        # ------------- Phase 1: q softmax (channels-first) + k/v + ctx -------------
        with ExitStack() as p1:
            ps_ctx = p1.enter_context(tc.tile_pool(name="ps_ctx", bufs=1, space="PSUM"))
            ctxa = ps_ctx.tile([128, CC, 129], F32, tag="ctxa", name="ctxa")

            p1i = p1.enter_context(ExitStack())
            lp1 = p1i.enter_context(tc.tile_pool(name="lp1", bufs=2))
            lpk = p1i.enter_context(tc.tile_pool(name="lpk", bufs=1))
            ps_q = p1i.enter_context(tc.tile_pool(name="ps_q", bufs=2, space="PSUM"))
            ps_m = p1i.enter_context(tc.tile_pool(name="ps_m", bufs=2, space="PSUM"))
            ps_k = p1i.enter_context(tc.tile_pool(name="ps_k", bufs=2, space="PSUM"))

            # persistent Ek/vT pair tiles (2 rotating pairs); the ones-columns
            # of vT are set once and never overwritten
            Ek2s = [lpk.tile([128, 2, 512], F8, tag=f"Ek2{i}", name=f"Ek2{i}")
                    for i in range(2)]
            # chunk blocks padded 129 -> 144 so the DoubleRow rhs outer stride
            # (2*288... the slot stride 576 and block step 144) is 16-aligned
            vT2s = [lpk.tile([128, 2, 576], F8, tag=f"vT2{i}", name=f"vT2{i}")
                    for i in range(2)]
            for i in range(2):
                nc.vector.memset(
                    vT2s[i][:].rearrange("p t (pr x) -> p t pr x", x=144)[:, :, :, 128:129],
                    1.0)

            # q-section for tile `ot` is emitted in 3 pieces interleaved into
            # the kv/ctx loop of tile ot-1 so the Eq-activation latency never
            # stalls PE: piece 0 = DMA + q matmuls for oc 0,1; piece 1 = exps
            # for oc 0,1 + q matmuls oc 2,3; piece 2 = exps oc 2,3 + per-head
            # sums (DoubleRow mask matmul into a spare qps-ring slice).
            qstate = {}

            def q_piece(ot, k):
                sl = slice(ot * 512, (ot + 1) * 512)
                if k == 0:
                    st = qstate[ot] = {}
                    st["z1c"] = lp1.tile([128, CC, 512], F8, tag="z1c", name="z1c")
                    nc.sync.dma_start(st["z1c"][:], z1_8d[:, :, sl])
                    st["z2c"] = lp1.tile([128, CC, 512], F8, tag="z2c", name="z2c")
                    nc.sync.dma_start(st["z2c"][:], z2_8d[:, :, sl])
                    st["qsmE"] = lp1.tile([128, CC, 512], F8, tag="qsmE", name="qsmE")
                    st["qp"] = []
                st = qstate[ot]
                if k in (0, 1):
                    for i in range(2):
                        oc = 2 * k + i
                        os_ = slice(oc * 128, (oc + 1) * 128)
                        qp = ps_q.tile([128, 512], F32, tag="qps", name="qp")
                        st["qp"].append(qp)
                        mm(qp[:], Wq8T[:, 0:2, os_], st["z1c"][:, 0:2, :],
                           start=True, stop=False, perf_mode=DR)
                        mm(qp[:], Wq8T[:, 2:4, os_], st["z1c"][:, 2:4, :],
                           start=False, stop=not bq_nz, perf_mode=DR)
                        if bq_nz:
                            mm(qp[:], bq_r[:, os_], ones_row[:],
                               start=False, stop=True)
                if k == 1:
                    for oc in (0, 1):
                        act(st["qsmE"][:, oc, :], st["qp"][oc][:], AFT.Exp,
                            scale=1.0 / SW)
                if k == 2:
                    for oc in (2, 3):
                        act(st["qsmE"][:, oc, :], st["qp"][oc][:], AFT.Exp,
                            scale=1.0 / SW)
                    sqt = ps_q.tile([128, 512], F32, tag="qps", name="sqt")
                    st["sqt"] = sqt
                    mm(sqt[0:16, :], hm8[:, 0:2, :], st["qsmE"][:, 0:2, :],
                       start=True, stop=False, perf_mode=DR, skip_group_check=True)
                    mm(sqt[0:16, :], hm8[:, 2:4, :], st["qsmE"][:, 2:4, :],
                       start=False, stop=True, perf_mode=DR, skip_group_check=True)
                    rqf = lp1.tile([8, 512], F32R, tag="rqf", name="rqf")
                    st["rqf"] = rqf
                    with nc.allow_low_precision(reason="f32r row for broadcast mm"):
                        nc.vector.reciprocal(rqf[:], sqt[0:8, :])

            q_piece(0, 0)
            late = load_late_consts()
            WrTb, W1gTb, W2gTb = late["WrTb"], late["W1gTb"], late["W2gTb"]
            u2ct, inv512, ivg8, g2c = (late["u2ct"], late["inv512"],
                                       late["ivg8"], late["g2c"])
            ones8p = late["ones8p"]
            br_c = late.get("br_c")
            w1bb_r = late.get("w1bb_r")
            g2b2_r = late.get("g2b2_r")
            be2_c = late.get("be2_c")
            for k in (1, 2):
                q_piece(0, k)

            for ot in range(NT):
                sl = slice(ot * 512, (ot + 1) * 512)
                st = qstate[ot]
                z2c, qsmE, rqf = st["z2c"], st["qsmE"], st["rqf"]
                for s in range(4):
                    ls = slice(s * 128, (s + 1) * 128)
                    slot = s % 2
                    pair = (ot * 2 + s // 2) % 2
                    Ek2, vT2 = Ek2s[pair], vT2s[pair]
                    pr = s
                    # qsm8 = qsmE * (64/Sq) broadcast per head
                    rqbt = ps_m.tile([128, 512], F32, tag="mps", name="rqbt")
                    mm(rqbt[:], hm64[:, pr, :], rqf[:],
                       start=True, stop=True)
                    tt(qsm8[:, pr, sl], qsmE[:, pr, :], rqbt[:],
                       AluOpType.mult)
                    kps = ps_k.tile([128, 512], F32, tag="kps", name="kps")
                    for p in (0, 2):
                        mm(kps[:], z2c[:, p:p + 2, ls], Wkv8T[:, p:p + 2, 0:512],
                           start=(p == 0), stop=(p == 2), perf_mode=DR)
                    vps = ps_m.tile([128, 512], F32, tag="mps", name="vps")
                    for p in (0, 2):
                        mm(vps[:], z2c[:, p:p + 2, ls], Wkv8T[:, p:p + 2, 512:1024],
                           start=(p == 0), stop=(p == 2), perf_mode=DR)
                    act(Ek2[:, slot, :], kps[:], AFT.Exp, scale=1.0 / SW)
                    vdst = vT2[:, slot, :].rearrange("p (pr x) -> p pr x", x=144)[:, :, 0:128]
                    vsrc = vps[:].rearrange("p (pr x) -> p pr x", x=128)
                    if s % 2 == 0:
                        nc.vector.tensor_copy(vdst, vsrc)
                    else:
                        act(vdst, vsrc, AFT.Copy)
                    if slot == 1:
                        first = (ot == 0 and s == 1)
                        last = (ot == NT - 1 and s == 3)
                        for pr2 in range(CC):
                            mm(ctxa[:, pr2, :], Ek2[:, :, pr2 * 128:(pr2 + 1) * 128],
                               vT2[:, :, pr2 * 144:pr2 * 144 + 129],
                               start=first, stop=last, perf_mode=DR,
                               skip_group_check=True)
                    if ot + 1 < NT and s < 3:
                        q_piece(ot + 1, s)
                if ot in qstate:
                    del qstate[ot]

            # finalize: normalize ctx rows, build W_combT = ctx_bd^T @ Wr^T in fp8
            p1i.close()
            with ExitStack() as fz:
                ft = fz.enter_context(tc.tile_pool(name="ft", bufs=1))
                ps_t = fz.enter_context(tc.tile_pool(name="ps_t", bufs=2, space="PSUM"))
                rs_l, cbd_l, tps_l, cT_l = [], [], [], []
                for pr in range(CC):
                    rs = ft.tile([128, 1], F32, tag=f"rs{pr}", name=f"rs{pr}")
                    nc.vector.reciprocal(rs[:], ctxa[:, pr, 128:129])
                    rs_l.append(rs)
                    cbd = ft.tile([128, 128], BF16, tag=f"cbd{pr}", name=f"cbd{pr}")
                    nc.vector.memset(cbd[:], 0.0)
                    cbd_l.append(cbd)
                for pr in range(CC):
                    ts(cbd_l[pr][0:64, 0:64], ctxa[0:64, pr, 0:64], rs_l[pr][0:64, :],
                       1.0 / SW, AluOpType.mult, AluOpType.mult)
                    ts(cbd_l[pr][64:128, 64:128], ctxa[64:128, pr, 64:128],
                       rs_l[pr][64:128, :], 1.0 / SW, AluOpType.mult, AluOpType.mult)
                for pr in range(CC):
                    tps = ps_t.tile([128, 128], BF16, tag="tps")
                    nc.tensor.transpose(tps[:], cbd_l[pr][:], identb[:])
                    tps_l.append(tps)
                    cT = ft.tile([128, 128], BF16, tag=f"cT{pr}", name=f"cT{pr}")
                    nc.vector.tensor_copy(cT[:], tps[:])
                    cT_l.append(cT)
                for pr in range(CC):
                    wcps = ps_t.tile([128, 512], F32, tag="wcps")
                    mm(wcps[:], cT_l[pr][:], WrTb[:, pr, :], start=True, stop=True)
                    act(WcT8[:, pr, :], wcps[:], AFT.Copy, scale=SC)

        # ------------- Phase 2: z = Wc qsm + z1, LN1, FFN, LN2 -------------
        # Software-pipelined: stage A (z + LN1 stats + xn) runs one tile ahead
        # of stage B (FFN + LN2 + output) so B's long FFN matmul stretch hides
        # A's LN1 latency chain and A(t+1)'s z matmuls hide B(t)'s LN2 tail.
        with ExitStack() as p2:
            lp2 = p2.enter_context(tc.tile_pool(name="lp2", bufs=2))
            lpx = p2.enter_context(tc.tile_pool(name="lpx", bufs=3))
            lpr = p2.enter_context(tc.tile_pool(name="lpr", bufs=2))
            lpe = p2.enter_context(tc.tile_pool(name="lpe", bufs=3))
            ps_z = p2.enter_context(tc.tile_pool(name="ps_z", bufs=2, space="PSUM"))
            ps_f = p2.enter_context(tc.tile_pool(name="ps_f", bufs=2, space="PSUM"))
            ps_f2 = p2.enter_context(tc.tile_pool(name="ps_f2", bufs=2, space="PSUM"))
            ps_row = p2.enter_context(tc.tile_pool(name="ps_row", bufs=2, space="PSUM"))

            def stage_a(lt):
                sl = slice(lt * 512, (lt + 1) * 512)
                z1r = lp2.tile([128, CC, 512], BF16, tag="z1r", name="z1r")
                nc.sync.dma_start(z1r[:], z1res_d[:, :, sl])
                rows = ps_row.tile([128, 512], F32, tag="rows", name="rows")

                zb = lp2.tile([128, CC, 512], BF16, tag="zb", name="zb")
                zsq = lp2.tile([128, CC, 512], F8, tag="zsq", name="zsq")
                for oc in range(CC):
                    os_ = slice(oc * 128, (oc + 1) * 128)
                    zps = ps_z.tile([128, 512], F32, tag="zps", name="zps")
                    mm(zps[:], WcT8[:, 0:2, os_], qsm8[:, 0:2, sl],
                       start=True, stop=False, perf_mode=DR)
                    mm(zps[:], WcT8[:, 2:4, os_], qsm8[:, 2:4, sl],
                       start=False, stop=True, perf_mode=DR)
                    stt(zb[:, oc, :], zps[:], 1.0 / (SC * SQ), z1r[:, oc, :],
                        AluOpType.mult, AluOpType.add)
                    if br_nz:
                        gp.tensor_scalar(zb[:, oc, :], zb[:, oc, :],
                                         br_c[:, oc:oc + 1], None, AluOpType.add)
                    act(zsq[:, oc, :], zb[:, oc, :], AFT.Square)
                for oc in range(CC):
                    mm(rows[0:1, :], inv512[:], zb[:, oc, :],
                       start=(oc == 0), stop=(oc == CC - 1), skip_group_check=True)
                # sum of squares via fp8 DoubleRow (ones lhsT); 1/512 is folded
                # into the variance row op below
                mm(rows[32:33, :], ones8p[:, 0:2, 0:1], zsq[:, 0:2, :],
                   start=True, stop=False, perf_mode=DR, skip_group_check=True)
                mm(rows[32:33, :], ones8p[:, 2:4, 0:1], zsq[:, 2:4, :],
                   start=False, stop=True, perf_mode=DR, skip_group_check=True)

                # LN1 rows: 1/sigma = exp(-0.5 ln(var+eps)) keeps every ACT op
                # in the same activation-table set (no table reloads)
                musq = lpr.tile([1, 512], BF16, tag="musq", name="musq")
                act(musq[:], rows[0:1, :], AFT.Square)
                varb = lpr.tile([1, 512], BF16, tag="varb", name="varb")
                stt(varb[:], rows[32:33, :], 1.0 / 512.0, musq[:],
                    AluOpType.mult, AluOpType.subtract)
                lnv = lpr.tile([1, 512], F32, tag="lnv", name="lnv")
                act(lnv[:], varb[:], AFT.Ln, bias=epsA[0:1, :])
                invbr = lpr.tile([1, 512], BF16, tag="invbr", name="invbr")
                act(invbr[:], lnv[:], AFT.Exp, scale=-0.5)
                numur = lpr.tile([1, 512], BF16, tag="numur", name="numur")
                stt(numur[:], rows[0:1, :], -1.0, invbr[:], AluOpType.mult,
                    AluOpType.mult)
                invsb = lp2.tile([128, 512], BF16, tag="invsb", name="invsb")
                gp.partition_broadcast(invsb[:], invbr[:])
                numub = lp2.tile([128, 512], BF16, tag="numub", name="numub")
                gp.partition_broadcast(numub[:], numur[:])

                xn = lpx.tile([128, CC, 512], BF16, tag="xn", name="xn")
                for oc in range(CC):
                    tt(xn[:, oc, :], zb[:, oc, :], invsb[:], AluOpType.mult)
                    tt(xn[:, oc, :], xn[:, oc, :], numub[:], AluOpType.add)
                return sl, rows, xn

            def b2_chunk(st, oc, s1, sq2):
                sl2, heh2, negm2b2 = st
                os_ = slice(oc * 128, (oc + 1) * 128)
                f2ps = ps_f2.tile([128, 512], F32, tag="f2ps", name="f2ps")
                for j in range(8):
                    mm(f2ps[:], W2gTb[:, j, os_], heh2[j // 4][:, j % 4, :],
                       start=(j == 0), stop=(j == 7 and not b2_nz))
                if b2_nz:
                    mm(f2ps[:], g2b2_r[:, os_], ones_row[:], start=False, stop=True)
                stt(s1[:, oc, :], negm2b2[:], g2c[:, oc:oc + 1], f2ps[:],
                    AluOpType.mult, AluOpType.add)
                gp.tensor_tensor(sq2[:, oc, :], s1[:, oc, :], s1[:, oc, :],
                                 AluOpType.mult)

            def b2_tail(st, s1, sq2):
                sl2, heh2, negm2b2 = st
                # variance row via fp8 DoubleRow (ivg8 = 1/g2^2 in col 0);
                # the 1/512 is folded into the Ln scale
                e2t = ps_f.tile([128, 512], F32, tag="fps", name="e2t")
                mm(e2t[0:1, :], ivg8[:, 0:2, 0:1], sq2[:, 0:2, :],
                   start=True, stop=False, perf_mode=DR, skip_group_check=True)
                mm(e2t[0:1, :], ivg8[:, 2:4, 0:1], sq2[:, 2:4, :],
                   start=False, stop=True, perf_mode=DR, skip_group_check=True)

                ln2v = lpr.tile([1, 512], F32, tag="ln2v", name="ln2v")
                act(ln2v[:], e2t[0:1, :], AFT.Ln, scale=1.0 / 512.0,
                    bias=epsA[0:1, :])
                inv2br = lpr.tile([1, 512], BF16, tag="inv2br", name="inv2br")
                act(inv2br[:], ln2v[:], AFT.Exp, scale=-0.5)
                invs2b = lp2.tile([128, 512], BF16, tag="invs2b", name="invs2b")
                gp.partition_broadcast(invs2b[:], inv2br[:])

                for oc in range(CC):
                    yo = lp2.tile([128, 512], BF16, tag=f"yo{oc}", name=f"yo{oc}")
                    tt(yo[:], s1[:, oc, :], invs2b[:], AluOpType.mult)
                    if be2_nz:
                        ts(yo[:], yo[:], be2_c[:, oc:oc + 1], None, AluOpType.add)
                    nc.sync.dma_start(outd[oc * 128:(oc + 1) * 128, sl2], yo[:])

            def stage_b(a_st, b_st):
                """FFN1+ELU for tile a_st, with the previous tile's FFN2
                oc-chunks interleaved between FFN1 j-pairs so PE always has
                independent matmuls while the ELU chain drains."""
                if b_st is not None:
                    s1 = lp2.tile([128, CC, 512], BF16, tag="s1", name="s1")
                    sq2 = lp2.tile([128, CC, 512], F8, tag="sq2", name="sq2")
                if a_st is None:
                    for oc in range(CC):
                        b2_chunk(b_st, oc, s1, sq2)
                    b2_tail(b_st, s1, sq2)
                    return None
                sl, rows, xn = a_st
                heh = [lp2.tile([128, 4, 512], BF16, tag=f"he{h}", name=f"he{h}")
                       for h in range(2)]
                for j in range(8):
                    fps = ps_f.tile([128, 512], F32, tag="fps", name="fps")
                    js = slice(j * 128, (j + 1) * 128)
                    for cc in range(CC):
                        mm(fps[:], W1gTb[:, cc, js], xn[:, cc, :],
                           start=(cc == 0), stop=(cc == CC - 1 and not b1_nz))
                    if b1_nz:
                        mm(fps[:], w1bb_r[:, js], ones_row[:], start=False, stop=True)
                    Eb = lpe.tile([128, 512], BF16, tag="Eb", name="Eb")
                    act(Eb[:], fps[:], AFT.Exp)
                    ts(Eb[:], Eb[:], 1.0, -1.0, AluOpType.min, AluOpType.add)
                    # elu in one DVE pass: max(h,0) + (min(exp(h),1)-1)
                    stt(heh[j // 4][:, j % 4, :], fps[:], 0.0, Eb[:],
                        AluOpType.max, AluOpType.add)
                    if b_st is not None and j % 2 == 1:
                        b2_chunk(b_st, j // 2, s1, sq2)
                for j in range(8):
                    mm(rows[64:65, :], u2ct[:, j:j + 1], heh[j // 4][:, j % 4, :],
                       start=(j == 0), stop=(j == 7), skip_group_check=True)
                negm2 = lpr.tile([1, 512], BF16, tag="negm2", name="negm2")
                ts(negm2[:], rows[64:65, :], -1.0, -B2MEAN_PLACEHOLDER,
                   AluOpType.mult, AluOpType.add)
                negm2b = lp2.tile([128, 512], BF16, tag="negm2b", name="negm2b")
                gp.partition_broadcast(negm2b[:], negm2[:])
                if b_st is not None:
                    b2_tail(b_st, s1, sq2)
                return sl, heh, negm2b

            pa, pb = None, None
            for lt in range(NT):
                cur = stage_a(lt)
                if pa is not None:
                    pb = stage_b(pa, pb)
                pa = cur
            pb = stage_b(pa, pb)
            stage_b(None, pb)

    nc.compile()
    return nc


def _prep_consts(Wq, bq, Wk, bk, Wv, bv, Wr, br, g1, be1, W1, b1, W2, b2, g2, be2):
    import ml_dtypes
    f = np.float32
    F8NP = ml_dtypes.float8_e4m3
    BFNP = ml_dtypes.bfloat16

    def chunkT(a, n):          # [n*128, m] -> [128, n, m]
        return np.ascontiguousarray(a.reshape(n, 128, -1).transpose(1, 0, 2))

    def colsT(v, n):           # [n*128] -> [128, n]
        return np.ascontiguousarray(v.reshape(n, 128).T)

    WqT = np.ascontiguousarray(Wq.T, dtype=f)
    WkvT = np.concatenate([Wk.T, Wv.T], axis=1).astype(f)
    WrT = np.ascontiguousarray(Wr.T, dtype=f)
    W1g = (W1 * g1[None, :]).astype(f)
    W1gT = np.ascontiguousarray(W1g.T)
    W2g = (W2 * g2[:, None]).astype(f)
    W2gT = np.ascontiguousarray(W2g.T)
    w1bb = (W1 @ be1 + b1).astype(f)
    u2 = (W2.sum(axis=0) / 512.0).astype(f)
    ivg = (1.0 / (g2 * g2)).astype(f)          # 1/512 folded into Ln scale
    b2mean = float(np.mean(b2))
    br_eff = (br + Wr @ bv).astype(f)
    ivg8 = np.zeros((128, CC, 16), dtype=f)
    ivg8[:, :, 0] = colsT(ivg, CC)
    ones8p = np.zeros((128, CC, 16), dtype=f)
    ones8p[:, :, 0] = 1.0

    # head mask: channel (cc, p) -> global k-channel cc*128+p -> head //64
    chan = (np.arange(CC)[None, :] * 128 + np.arange(128)[:, None])  # [128, CC]
    head = chan // DK                                                # [128, CC]
    hm8 = np.zeros((128, CC, 16), dtype=f)   # padded to 16 cols for DoubleRow
    for hh in range(8):
        hm8[:, :, hh] = (head == hh)
    hm64 = np.zeros((8, CC, 128), dtype=f)
    for pr in range(CC):
        for hh in range(8):
            hm64[hh, pr, :] = 64.0 * (head[:, pr] == hh)

    consts = {
        "Wq8T": chunkT(WqT * SW, CC).astype(F8NP),
        "Wkv8T": chunkT(WkvT * SW, CC).astype(F8NP),
        "hm8": hm8.astype(F8NP),
        "hm64": hm64,
        "WrTb": chunkT(WrT, CC).astype(BFNP),
        "W1gTb": chunkT(W1gT, CC).astype(BFNP),
        "W2gTb": chunkT(W2gT, 8).astype(BFNP),
        "u2ct": colsT(u2, 8).astype(BFNP),
        "inv512": np.full((128, 1), 1.0 / 512.0, dtype=f).astype(BFNP),
        "ivg8": ivg8.astype(F8NP),
        "ones8p": ones8p.astype(F8NP),
        "g2c": colsT(g2.astype(f), CC),
        "identb": np.eye(128, dtype=f).astype(BFNP),
        "epsA": np.full((1, 1), EPS, dtype=f),
        "ones_row": np.ones((1, 512), dtype=f),
        "bq_r": bq.reshape(1, 512).astype(f),
        "br_c": colsT(br_eff, CC),
        "w1bb_r": w1bb.reshape(1, 1024).astype(f),
        "g2b2_r": (g2 * b2).reshape(1, 512).astype(f),
        "be2_c": colsT(be2.astype(f), CC),
    }
    flags = (bool(np.any(bq)), bool(np.any(br_eff)), bool(np.any(w1bb)),
             bool(np.any(b2)), bool(np.any(be2)))
    return consts, b2mean, flags


def kernel(**inputs):
    global LAST_RESULT, B2MEAN_PLACEHOLDER
    import ml_dtypes
    F8NP = ml_dtypes.float8_e4m3
    BFNP = ml_dtypes.bfloat16
    z1 = np.asarray(inputs["z1"], dtype=np.float32)
    z2 = np.asarray(inputs["z2"], dtype=np.float32)
    consts, b2mean, flags = _prep_consts(
        *[np.asarray(inputs[k], dtype=np.float32) for k in
          ["Wq", "bq", "Wk", "bk", "Wv", "bv", "Wr", "br", "g1", "be1",
           "W1", "b1", "W2", "b2", "g2", "be2"]])

    key = ("prog", flags, round(b2mean * 1e9))
    if key not in _CACHE:
        B2MEAN_PLACEHOLDER = b2mean
        _CACHE.clear()
        _CACHE[key] = _build_program(flags)
    nc = _CACHE[key]

    def rearr(a):             # [C, L] -> [128, CC, L]
        return np.ascontiguousarray(a.reshape(CC, 128, L).transpose(1, 0, 2))

    in_maps = []
    for b in range(B):
        m = dict(consts)
        m["z1_8"] = rearr(z1[b]).astype(F8NP)
        m["z2_8"] = rearr(z2[b]).astype(F8NP)
        m["z1res"] = rearr(z1[b]).astype(BFNP)
        in_maps.append(m)

    import os
    trace = bool(int(os.environ.get("KERNEL_TRACE", "0")))
    res = run_bass_kernel_spmd(nc, in_maps, list(range(B)), trace=trace)
    LAST_RESULT = res
    out = np.stack([np.asarray(res.results[b]["out"]).astype(np.float32)
                    for b in range(B)], axis=0)
    return out


B2MEAN_PLACEHOLDER = 0.0


# revision 66
# speedup vs baseline: 3.0606x; 1.0409x over previous
"""CACombiner Trainium2 kernel: conv-projected efficient attention + FFN.

Data-parallel over batch: 8 batch elements -> 8 NeuronCores, identical SPMD
program per core. Attention path (q/k/v projections, ctx, and the fused
Wr@ctx reprojection) runs in fp8e4 with DoubleRow matmuls (2 k-tiles per
instruction, 0.5 cycles/row); the FFN runs in bf16. LayerNorms are fused:
LN1 mean/var via PE row-matmuls + gpsimd partition-broadcast, LN2 likewise
with g2/b2 folded on host.
"""
import sys
sys.path.insert(0, "/opt/trn_rl_repo")
from contextlib import ExitStack

import numpy as np

import concourse.bass as bass
import concourse.tile as tile
from concourse import mybir, bacc
from concourse.bass_utils import run_bass_kernel_spmd
from concourse.alu_op_type import AluOpType

F32 = mybir.dt.float32
F32R = mybir.dt.float32r
BF16 = mybir.dt.bfloat16
F8 = mybir.dt.float8e4
AFT = mybir.ActivationFunctionType
DR = mybir.MatmulPerfMode.DoubleRow

# Every activation this kernel uses (Exp, Ln, Relu, Copy, Square, Identity)
# lives together in one hardware activation-table set. The default chooser
# picks the first set containing each function, which alternates sets and
# inserts a 1.3us LoadActFuncSet per switch. Narrow the chooser's view so the
# all-inclusive set is the unique provider (names/indexes preserved, so the
# emitted act_func_set_id still refers to the true table).
_OUR_FUNCS = {AFT.Exp, AFT.Ln, AFT.Relu, AFT.Copy, AFT.Square, AFT.Identity}


def _patch_act_tables():
    import concourse.hw_specs as hw_specs
    import concourse.bacc as bacc_mod
    orig = hw_specs.get_activation_tables
    if getattr(hw_specs, "_cac_patched", False):
        return

    def patched(arch):
        t = orig(arch)
        keep = None
        for name, s in t.items():
            if _OUR_FUNCS <= s:
                keep = name
                break
        if keep is None:
            return t
        return {name: (s if name == keep else s - _OUR_FUNCS)
                for name, s in t.items()}

    hw_specs.get_activation_tables = patched
    bacc_mod.get_activation_tables = patched
    hw_specs._cac_patched = True

B, C, L = 8, 512, 4096
H, DK = 8, 64
EPS = 1e-5
CC = C // 128            # 4 channel chunks
NT = L // 512            # 8 outer l-tiles
SW = 32.0                # fp8 scale for Wq/Wk/Wv
SQ = 64.0                # fp8 scale for softmaxed q
SC = 256.0               # fp8 scale for W_comb = Wr @ ctx

_CACHE = {}
LAST_RESULT = None


def _build_program(flags):
    bq_nz, br_nz, b1_nz, b2_nz, be2_nz = flags
    _patch_act_tables()
    nc = bacc.Bacc("TRN2", target_bir_lowering=False, debug=False)

    def din(name, shape, dtype):
        return nc.dram_tensor(name, list(shape), dtype, kind="ExternalInput").ap()

    z1_8d = din("z1_8", (128, CC, L), F8)
    z2_8d = din("z2_8", (128, CC, L), F8)
    z1res_d = din("z1res", (128, CC, L), BF16)
    Wq8T_d = din("Wq8T", (128, CC, 512), F8)
    Wkv8T_d = din("Wkv8T", (128, CC, 1024), F8)
    hm8_d = din("hm8", (128, CC, 16), F8)
    hm64_d = din("hm64", (8, CC, 128), F32R)
    WrTb_d = din("WrTb", (128, CC, 512), BF16)
    W1gTb_d = din("W1gTb", (128, CC, 1024), BF16)
    W2gTb_d = din("W2gTb", (128, 8, 512), BF16)
    u2ct_d = din("u2ct", (128, 8), BF16)
    inv512_d = din("inv512", (128, 1), BF16)
    ivg8_d = din("ivg8", (128, CC, 16), F8)
    ones8p_d = din("ones8p", (128, CC, 16), F8)
    g2c_d = din("g2c", (128, CC), F32)
    identb_d = din("identb", (128, 128), BF16)
    eps_d = din("epsA", (1, 1), F32)
    ones_row_d = din("ones_row", (1, 512), F32R)
    bq_r_d = din("bq_r", (1, 512), F32R)
    br_c_d = din("br_c", (128, CC), F32)
    w1bb_r_d = din("w1bb_r", (1, 1024), F32R)
    g2b2_r_d = din("g2b2_r", (1, 512), F32R)
    be2_c_d = din("be2_c", (128, CC), F32)
    outd = nc.dram_tensor("out", [C, L], BF16, kind="ExternalOutput").ap()

    mm = nc.tensor.matmul
    tt = nc.vector.tensor_tensor
    ts = nc.vector.tensor_scalar
    stt = nc.vector.scalar_tensor_tensor
    act = nc.scalar.activation
    gp = nc.gpsimd

    with tile.TileContext(nc) as tc, ExitStack() as ctx:
        cpool = ctx.enter_context(tc.tile_pool(name="consts", bufs=1))

        def const_tile(shape, dtype, src, tag):
            t = cpool.tile(list(shape), dtype, tag=tag, name=tag)
            nc.sync.dma_start(t[:], src)
            return t

        # phase-1 weights first so the first q matmuls aren't queued behind
        # the big FFN weight transfers; the rest loads during phase 1
        Wq8T = const_tile((128, CC, 512), F8, Wq8T_d, "Wq8T")
        Wkv8T = const_tile((128, CC, 1024), F8, Wkv8T_d, "Wkv8T")
        hm8 = const_tile((128, CC, 16), F8, hm8_d, "hm8")
        hm64 = const_tile((8, CC, 128), F32R, hm64_d, "hm64")
        identb = const_tile((128, 128), BF16, identb_d, "identb")
        epsA = const_tile((1, 1), F32, eps_d, "epsA")
        ones_row = const_tile((1, 512), F32R, ones_row_d, "ones_row")
        if bq_nz:
            bq_r = const_tile((1, 512), F32R, bq_r_d, "bq_r")

        def load_late_consts():
            c = {}
            c["WrTb"] = const_tile((128, CC, 512), BF16, WrTb_d, "WrTb")
            c["W1gTb"] = const_tile((128, CC, 1024), BF16, W1gTb_d, "W1gTb")
            c["W2gTb"] = const_tile((128, 8, 512), BF16, W2gTb_d, "W2gTb")
            c["u2ct"] = const_tile((128, 8), BF16, u2ct_d, "u2ct")
            c["inv512"] = const_tile((128, 1), BF16, inv512_d, "inv512")
            c["ivg8"] = const_tile((128, CC, 16), F8, ivg8_d, "ivg8")
            c["ones8p"] = const_tile((128, CC, 16), F8, ones8p_d, "ones8p")
            c["g2c"] = const_tile((128, CC), F32, g2c_d, "g2c")
            if br_nz:
                c["br_c"] = const_tile((128, CC), F32, br_c_d, "br_c")
            if b1_nz:
                c["w1bb_r"] = const_tile((1, 1024), F32R, w1bb_r_d, "w1bb_r")
            if b2_nz:
                c["g2b2_r"] = const_tile((1, 512), F32R, g2b2_r_d, "g2b2_r")
            if be2_nz:
                c["be2_c"] = const_tile((128, CC), F32, be2_c_d, "be2_c")
            return c

        # persistent across phases
        qsm8 = cpool.tile([128, CC, L], F8, tag="qsm8", name="qsm8")
        WcT8 = cpool.tile([128, CC, 512], F8, tag="WcT8", name="WcT8")

        # ------------- Phase 1: q softmax (channels-first) + k/v + ctx -------------
        with ExitStack() as p1:
            ps_ctx = p1.enter_context(tc.tile_pool(name="ps_ctx", bufs=1, space="PSUM"))
            ctxa = ps_ctx.tile([128, CC, 129], F32, tag="ctxa", name="ctxa")

            p1i = p1.enter_context(ExitStack())
            lp1 = p1i.enter_context(tc.tile_pool(name="lp1", bufs=3))
            lpk = p1i.enter_context(tc.tile_pool(name="lpk", bufs=1))
            ps_q = p1i.enter_context(tc.tile_pool(name="ps_q", bufs=2, space="PSUM"))
            ps_m = p1i.enter_context(tc.tile_pool(name="ps_m", bufs=2, space="PSUM"))
            ps_k = p1i.enter_context(tc.tile_pool(name="ps_k", bufs=2, space="PSUM"))

            # persistent Ek/vT pair tiles (2 rotating pairs); the ones-columns
            # of vT are set once and never overwritten
            Ek2s = [lpk.tile([128, 2, 512], F8, tag=f"Ek2{i}", name=f"Ek2{i}")
                    for i in range(2)]
            # chunk blocks padded 129 -> 144 so the DoubleRow rhs outer stride
            # (2*288... the slot stride 576 and block step 144) is 16-aligned
            vT2s = [lpk.tile([128, 2, 576], F8, tag=f"vT2{i}", name=f"vT2{i}")
                    for i in range(2)]
            for i in range(2):
                nc.vector.memset(
                    vT2s[i][:].rearrange("p t (pr x) -> p t pr x", x=144)[:, :, :, 128:129],
                    1.0)

            # q-section for tile `ot` is emitted in 3 pieces interleaved into
            # the kv/ctx loop of tile ot-1 so the Eq-activation latency never
            # stalls PE: piece 0 = DMA + q matmuls for oc 0,1; piece 1 = exps
            # for oc 0,1 + q matmuls oc 2,3; piece 2 = exps oc 2,3 + per-head
            # sums (DoubleRow mask matmul into a spare qps-ring slice).
            qstate = {}

            def q_piece(ot, k):
                sl = slice(ot * 512, (ot + 1) * 512)
                if k == 0:
                    st = qstate[ot] = {}
                    st["z1c"] = lp1.tile([128, CC, 512], F8, tag="z1c", name="z1c")
                    nc.sync.dma_start(st["z1c"][:], z1_8d[:, :, sl])
                    st["z2c"] = lp1.tile([128, CC, 512], F8, tag="z2c", name="z2c")
                    nc.sync.dma_start(st["z2c"][:], z2_8d[:, :, sl])
                    st["qsmE"] = lp1.tile([128, CC, 512], F8, tag="qsmE", name="qsmE")
                    st["qp"] = []
                st = qstate[ot]
                if k in (0, 1):
                    for i in range(2):
                        oc = 2 * k + i
                        os_ = slice(oc * 128, (oc + 1) * 128)
                        qp = ps_q.tile([128, 512], F32, tag="qps", name="qp")
                        st["qp"].append(qp)
                        mm(qp[:], Wq8T[:, 0:2, os_], st["z1c"][:, 0:2, :],
                           start=True, stop=False, perf_mode=DR)
                        mm(qp[:], Wq8T[:, 2:4, os_], st["z1c"][:, 2:4, :],
                           start=False, stop=not bq_nz, perf_mode=DR)
                        if bq_nz:
                            mm(qp[:], bq_r[:, os_], ones_row[:],
                               start=False, stop=True)
                if k == 1:
                    for oc in (0, 1):
                        act(st["qsmE"][:, oc, :], st["qp"][oc][:], AFT.Exp,
                            scale=1.0 / SW)
                if k == 2:
                    for oc in (2, 3):
                        act(st["qsmE"][:, oc, :], st["qp"][oc][:], AFT.Exp,
                            scale=1.0 / SW)
                    sqt = ps_q.tile([128, 512], F32, tag="qps", name="sqt")
                    st["sqt"] = sqt
                    mm(sqt[0:16, :], hm8[:, 0:2, :], st["qsmE"][:, 0:2, :],
                       start=True, stop=False, perf_mode=DR, skip_group_check=True)
                    mm(sqt[0:16, :], hm8[:, 2:4, :], st["qsmE"][:, 2:4, :],
                       start=False, stop=True, perf_mode=DR, skip_group_check=True)
                    rqf = lp1.tile([8, 512], F32R, tag="rqf", name="rqf")
                    st["rqf"] = rqf
                    with nc.allow_low_precision(reason="f32r row for broadcast mm"):
                        nc.vector.reciprocal(rqf[:], sqt[0:8, :])

            q_piece(0, 0)
            late = load_late_consts()
            WrTb, W1gTb, W2gTb = late["WrTb"], late["W1gTb"], late["W2gTb"]
            u2ct, inv512, ivg8, g2c = (late["u2ct"], late["inv512"],
                                       late["ivg8"], late["g2c"])
            ones8p = late["ones8p"]
            br_c = late.get("br_c")
            w1bb_r = late.get("w1bb_r")
            g2b2_r = late.get("g2b2_r")
            be2_c = late.get("be2_c")
            for k in (1, 2):
                q_piece(0, k)

            for ot in range(NT):
                sl = slice(ot * 512, (ot + 1) * 512)
                st = qstate[ot]
                z2c, qsmE, rqf = st["z2c"], st["qsmE"], st["rqf"]
                for s in range(4):
                    ls = slice(s * 128, (s + 1) * 128)
                    slot = s % 2
                    pair = (ot * 2 + s // 2) % 2
                    Ek2, vT2 = Ek2s[pair], vT2s[pair]
                    pr = s
                    # qsm8 = qsmE * (64/Sq) broadcast per head
                    rqbt = ps_m.tile([128, 512], F32, tag="mps", name="rqbt")
                    mm(rqbt[:], hm64[:, pr, :], rqf[:],
                       start=True, stop=True)
                    tt(qsm8[:, pr, sl], qsmE[:, pr, :], rqbt[:],
                       AluOpType.mult)
                    kps = ps_k.tile([128, 512], F32, tag="kps", name="kps")
                    for p in (0, 2):
                        mm(kps[:], z2c[:, p:p + 2, ls], Wkv8T[:, p:p + 2, 0:512],
                           start=(p == 0), stop=(p == 2), perf_mode=DR)
                    vps = ps_m.tile([128, 512], F32, tag="mps", name="vps")
                    for p in (0, 2):
                        mm(vps[:], z2c[:, p:p + 2, ls], Wkv8T[:, p:p + 2, 512:1024],
                           start=(p == 0), stop=(p == 2), perf_mode=DR)
                    act(Ek2[:, slot, :], kps[:], AFT.Exp, scale=1.0 / SW)
                    vdst = vT2[:, slot, :].rearrange("p (pr x) -> p pr x", x=144)[:, :, 0:128]
                    vsrc = vps[:].rearrange("p (pr x) -> p pr x", x=128)
                    if s == 3:
                        act(vdst, vsrc, AFT.Copy)
                    else:
                        nc.vector.tensor_copy(vdst, vsrc)
                    if slot == 1:
                        first = (ot == 0 and s == 1)
                        last = (ot == NT - 1 and s == 3)
                        for pr2 in range(CC):
                            mm(ctxa[:, pr2, :], Ek2[:, :, pr2 * 128:(pr2 + 1) * 128],
                               vT2[:, :, pr2 * 144:pr2 * 144 + 129],
                               start=first, stop=last, perf_mode=DR,
                               skip_group_check=True)
                    if ot + 1 < NT and s < 3:
                        q_piece(ot + 1, s)
                if ot in qstate:
                    del qstate[ot]

            # finalize: normalize ctx rows, build W_combT = ctx_bd^T @ Wr^T in fp8
            p1i.close()
            with ExitStack() as fz:
                ft = fz.enter_context(tc.tile_pool(name="ft", bufs=1))
                ps_t = fz.enter_context(tc.tile_pool(name="ps_t", bufs=2, space="PSUM"))
                rs_l, cbd_l, tps_l, cT_l = [], [], [], []
                for pr in range(CC):
                    rs = ft.tile([128, 1], F32, tag=f"rs{pr}", name=f"rs{pr}")
                    nc.vector.reciprocal(rs[:], ctxa[:, pr, 128:129])
                    rs_l.append(rs)
                    cbd = ft.tile([128, 128], BF16, tag=f"cbd{pr}", name=f"cbd{pr}")
                    nc.vector.memset(cbd[:], 0.0)
                    cbd_l.append(cbd)
                for pr in range(CC):
                    ts(cbd_l[pr][0:64, 0:64], ctxa[0:64, pr, 0:64], rs_l[pr][0:64, :],
                       1.0 / SW, AluOpType.mult, AluOpType.mult)
                    ts(cbd_l[pr][64:128, 64:128], ctxa[64:128, pr, 64:128],
                       rs_l[pr][64:128, :], 1.0 / SW, AluOpType.mult, AluOpType.mult)
                for pr in range(CC):
                    tps = ps_t.tile([128, 128], BF16, tag="tps")
                    nc.tensor.transpose(tps[:], cbd_l[pr][:], identb[:])
                    tps_l.append(tps)
                    cT = ft.tile([128, 128], BF16, tag=f"cT{pr}", name=f"cT{pr}")
                    nc.vector.tensor_copy(cT[:], tps[:])
                    cT_l.append(cT)
                for pr in range(CC):
                    wcps = ps_t.tile([128, 512], F32, tag="wcps")
                    mm(wcps[:], cT_l[pr][:], WrTb[:, pr, :], start=True, stop=True)
                    act(WcT8[:, pr, :], wcps[:], AFT.Copy, scale=SC)

        # ------------- Phase 2: z = Wc qsm + z1, LN1, FFN, LN2 -------------
        # Software-pipelined: stage A (z + LN1 stats + xn) runs one tile ahead
        # of stage B (FFN + LN2 + output) so B's long FFN matmul stretch hides
        # A's LN1 latency chain and A(t+1)'s z matmuls hide B(t)'s LN2 tail.
        with ExitStack() as p2:
            lp2 = p2.enter_context(tc.tile_pool(name="lp2", bufs=2))
            lpx = p2.enter_context(tc.tile_pool(name="lpx", bufs=3))
            lpr = p2.enter_context(tc.tile_pool(name="lpr", bufs=2))
            lpe = p2.enter_context(tc.tile_pool(name="lpe", bufs=3))
            ps_z = p2.enter_context(tc.tile_pool(name="ps_z", bufs=2, space="PSUM"))
            ps_f = p2.enter_context(tc.tile_pool(name="ps_f", bufs=2, space="PSUM"))
            ps_f2 = p2.enter_context(tc.tile_pool(name="ps_f2", bufs=2, space="PSUM"))
            ps_row = p2.enter_context(tc.tile_pool(name="ps_row", bufs=2, space="PSUM"))

            def stage_a(lt):
                sl = slice(lt * 512, (lt + 1) * 512)
                z1r = lp2.tile([128, CC, 512], BF16, tag="z1r", name="z1r")
                nc.sync.dma_start(z1r[:], z1res_d[:, :, sl])
                rows = ps_row.tile([128, 512], F32, tag="rows", name="rows")

                zb = lp2.tile([128, CC, 512], BF16, tag="zb", name="zb")
                zsq = lp2.tile([128, CC, 512], F8, tag="zsq", name="zsq")
                for oc in range(CC):
                    os_ = slice(oc * 128, (oc + 1) * 128)
                    zps = ps_z.tile([128, 512], F32, tag="zps", name="zps")
                    mm(zps[:], WcT8[:, 0:2, os_], qsm8[:, 0:2, sl],
                       start=True, stop=False, perf_mode=DR)
                    mm(zps[:], WcT8[:, 2:4, os_], qsm8[:, 2:4, sl],
                       start=False, stop=True, perf_mode=DR)
                    stt(zb[:, oc, :], zps[:], 1.0 / (SC * SQ), z1r[:, oc, :],
                        AluOpType.mult, AluOpType.add)
                    if br_nz:
                        gp.tensor_scalar(zb[:, oc, :], zb[:, oc, :],
                                         br_c[:, oc:oc + 1], None, AluOpType.add)
                    act(zsq[:, oc, :], zb[:, oc, :], AFT.Square)
                # mean row at partition 32 (bf16 matmul may target 32); the
                # fp8 DoubleRow square-sum must target partition 0
                for oc in range(CC):
                    mm(rows[32:33, :], inv512[:], zb[:, oc, :],
                       start=(oc == 0), stop=(oc == CC - 1), skip_group_check=True)
                mm(rows[0:1, :], ones8p[:, 0:2, 0:1], zsq[:, 0:2, :],
                   start=True, stop=False, perf_mode=DR, skip_group_check=True)
                mm(rows[0:1, :], ones8p[:, 2:4, 0:1], zsq[:, 2:4, :],
                   start=False, stop=True, perf_mode=DR, skip_group_check=True)

                # LN1 rows: 1/sigma = exp(-0.5 ln(var+eps)) keeps every ACT op
                # in the same activation-table set (no table reloads)
                musq = lpr.tile([1, 512], BF16, tag="musq", name="musq")
                act(musq[:], rows[32:33, :], AFT.Square)
                varb = lpr.tile([1, 512], BF16, tag="varb", name="varb")
                stt(varb[:], rows[0:1, :], 1.0 / 512.0, musq[:],
                    AluOpType.mult, AluOpType.subtract)
                lnv = lpr.tile([1, 512], F32, tag="lnv", name="lnv")
                act(lnv[:], varb[:], AFT.Ln, bias=epsA[0:1, :])
                invbr = lpr.tile([1, 512], BF16, tag="invbr", name="invbr")
                act(invbr[:], lnv[:], AFT.Exp, scale=-0.5)
                numur = lpr.tile([1, 512], BF16, tag="numur", name="numur")
                stt(numur[:], rows[32:33, :], -1.0, invbr[:], AluOpType.mult,
                    AluOpType.mult)
                invsb = lp2.tile([128, 512], BF16, tag="invsb", name="invsb")
                gp.partition_broadcast(invsb[:], invbr[:])
                numub = lp2.tile([128, 512], BF16, tag="numub", name="numub")
                gp.partition_broadcast(numub[:], numur[:])

                xn = lpx.tile([128, CC, 512], BF16, tag="xn", name="xn")
                for oc in range(CC):
                    eng = gp if oc == 3 else nc.vector
                    eng.tensor_tensor(xn[:, oc, :], zb[:, oc, :], invsb[:],
                                      AluOpType.mult)
                    eng.tensor_tensor(xn[:, oc, :], xn[:, oc, :], numub[:],
                                      AluOpType.add)
                return sl, rows, xn

            def b2_chunk(st, oc, s1, sq2):
                sl2, heh2, negm2b2 = st
                os_ = slice(oc * 128, (oc + 1) * 128)
                f2ps = ps_f2.tile([128, 512], F32, tag="f2ps", name="f2ps")
                for j in range(8):
                    mm(f2ps[:], W2gTb[:, j, os_], heh2[j // 4][:, j % 4, :],
                       start=(j == 0), stop=(j == 7 and not b2_nz))
                if b2_nz:
                    mm(f2ps[:], g2b2_r[:, os_], ones_row[:], start=False, stop=True)
                stt(s1[:, oc, :], negm2b2[:], g2c[:, oc:oc + 1], f2ps[:],
                    AluOpType.mult, AluOpType.add)
                gp.tensor_tensor(sq2[:, oc, :], s1[:, oc, :], s1[:, oc, :],
                                 AluOpType.mult)

            def b2_tail(st, s1, sq2):
                sl2, heh2, negm2b2 = st
                # variance row via fp8 DoubleRow (ivg8 = 1/g2^2 in col 0);
                # the 1/512 is folded into the Ln scale
                e2t = ps_f.tile([128, 512], F32, tag="fps", name="e2t")
                mm(e2t[0:1, :], ivg8[:, 0:2, 0:1], sq2[:, 0:2, :],
                   start=True, stop=False, perf_mode=DR, skip_group_check=True)
                mm(e2t[0:1, :], ivg8[:, 2:4, 0:1], sq2[:, 2:4, :],
                   start=False, stop=True, perf_mode=DR, skip_group_check=True)

                ln2v = lpr.tile([1, 512], F32, tag="ln2v", name="ln2v")
                act(ln2v[:], e2t[0:1, :], AFT.Ln, scale=1.0 / 512.0,
                    bias=epsA[0:1, :])
                inv2br = lpr.tile([1, 512], BF16, tag="inv2br", name="inv2br")
                act(inv2br[:], ln2v[:], AFT.Exp, scale=-0.5)
                invs2b = lp2.tile([128, 512], BF16, tag="invs2b", name="invs2b")
                gp.partition_broadcast(invs2b[:], inv2br[:])

                for oc in range(CC):
                    yo = lp2.tile([128, 512], BF16, tag=f"yo{oc}", name=f"yo{oc}")
                    eng = gp if oc % 2 == 0 else nc.vector
                    eng.tensor_tensor(yo[:], s1[:, oc, :], invs2b[:],
                                      AluOpType.mult)
                    if be2_nz:
                        ts(yo[:], yo[:], be2_c[:, oc:oc + 1], None, AluOpType.add)
                    nc.sync.dma_start(outd[oc * 128:(oc + 1) * 128, sl2], yo[:])

            def stage_b(a_st, b_st):
                """FFN1+ELU for tile a_st, with the previous tile's FFN2
                oc-chunks interleaved between FFN1 j-pairs so PE always has
                independent matmuls while the ELU chain drains."""
                if b_st is not None:
                    s1 = lp2.tile([128, CC, 512], BF16, tag="s1", name="s1")
                    sq2 = lp2.tile([128, CC, 512], F8, tag="sq2", name="sq2")
                if a_st is None:
                    for oc in range(CC):
                        b2_chunk(b_st, oc, s1, sq2)
                    b2_tail(b_st, s1, sq2)
                    return None
                sl, rows, xn = a_st
                heh = [lp2.tile([128, 4, 512], BF16, tag=f"he{h}", name=f"he{h}")
                       for h in range(2)]
                for j in range(8):
                    fps = ps_f.tile([128, 512], F32, tag="fps", name="fps")
                    js = slice(j * 128, (j + 1) * 128)
                    for cc in range(CC):
                        mm(fps[:], W1gTb[:, cc, js], xn[:, cc, :],
                           start=(cc == 0), stop=(cc == CC - 1 and not b1_nz))
                    if b1_nz:
                        mm(fps[:], w1bb_r[:, js], ones_row[:], start=False, stop=True)
                    Eb = lpe.tile([128, 512], BF16, tag="Eb", name="Eb")
                    act(Eb[:], fps[:], AFT.Exp)
                    ts(Eb[:], Eb[:], 1.0, -1.0, AluOpType.min, AluOpType.add)
                    if j % 2 == 0:
                        # elu in one DVE pass: max(h,0) + (min(exp(h),1)-1)
                        stt(heh[j // 4][:, j % 4, :], fps[:], 0.0, Eb[:],
                            AluOpType.max, AluOpType.add)
                    else:
                        hp = lpe.tile([128, 512], BF16, tag="hp", name="hp")
                        act(hp[:], fps[:], AFT.Relu)
                        tt(heh[j // 4][:, j % 4, :], hp[:], Eb[:], AluOpType.add)
                    if b_st is not None and j % 2 == 1:
                        b2_chunk(b_st, j // 2, s1, sq2)
                for j in range(8):
                    mm(rows[64:65, :], u2ct[:, j:j + 1], heh[j // 4][:, j % 4, :],
                       start=(j == 0), stop=(j == 7), skip_group_check=True)
                negm2 = lpr.tile([1, 512], BF16, tag="negm2", name="negm2")
                ts(negm2[:], rows[64:65, :], -1.0, -B2MEAN_PLACEHOLDER,
                   AluOpType.mult, AluOpType.add)
                negm2b = lp2.tile([128, 512], BF16, tag="negm2b", name="negm2b")
                gp.partition_broadcast(negm2b[:], negm2[:])
                if b_st is not None:
                    b2_tail(b_st, s1, sq2)
                return sl, heh, negm2b

            pa, pb = None, None
            for lt in range(NT):
                cur = stage_a(lt)
                if pa is not None:
                    pb = stage_b(pa, pb)
                pa = cur
            pb = stage_b(pa, pb)
            stage_b(None, pb)

    nc.compile()
    return nc


def _prep_consts(Wq, bq, Wk, bk, Wv, bv, Wr, br, g1, be1, W1, b1, W2, b2, g2, be2):
    import ml_dtypes
    f = np.float32
    F8NP = ml_dtypes.float8_e4m3
    BFNP = ml_dtypes.bfloat16

    def chunkT(a, n):          # [n*128, m] -> [128, n, m]
        return np.ascontiguousarray(a.reshape(n, 128, -1).transpose(1, 0, 2))

    def colsT(v, n):           # [n*128] -> [128, n]
        return np.ascontiguousarray(v.reshape(n, 128).T)

    WqT = np.ascontiguousarray(Wq.T, dtype=f)
    WkvT = np.concatenate([Wk.T, Wv.T], axis=1).astype(f)
    WrT = np.ascontiguousarray(Wr.T, dtype=f)
    W1g = (W1 * g1[None, :]).astype(f)
    W1gT = np.ascontiguousarray(W1g.T)
    W2g = (W2 * g2[:, None]).astype(f)
    W2gT = np.ascontiguousarray(W2g.T)
    w1bb = (W1 @ be1 + b1).astype(f)
    u2 = (W2.sum(axis=0) / 512.0).astype(f)
    ivg = (1.0 / (g2 * g2)).astype(f)          # 1/512 folded into Ln scale
    b2mean = float(np.mean(b2))
    br_eff = (br + Wr @ bv).astype(f)
    ivg8 = np.zeros((128, CC, 16), dtype=f)
    ivg8[:, :, 0] = colsT(ivg, CC)
    ones8p = np.zeros((128, CC, 16), dtype=f)
    ones8p[:, :, 0] = 1.0

    # head mask: channel (cc, p) -> global k-channel cc*128+p -> head //64
    chan = (np.arange(CC)[None, :] * 128 + np.arange(128)[:, None])  # [128, CC]
    head = chan // DK                                                # [128, CC]
    hm8 = np.zeros((128, CC, 16), dtype=f)   # padded to 16 cols for DoubleRow
    for hh in range(8):
        hm8[:, :, hh] = (head == hh)
    hm64 = np.zeros((8, CC, 128), dtype=f)
    for pr in range(CC):
        for hh in range(8):
            hm64[hh, pr, :] = 64.0 * (head[:, pr] == hh)

    consts = {
        "Wq8T": chunkT(WqT * SW, CC).astype(F8NP),
        "Wkv8T": chunkT(WkvT * SW, CC).astype(F8NP),
        "hm8": hm8.astype(F8NP),
        "hm64": hm64,
        "WrTb": chunkT(WrT, CC).astype(BFNP),
        "W1gTb": chunkT(W1gT, CC).astype(BFNP),
        "W2gTb": chunkT(W2gT, 8).astype(BFNP),
        "u2ct": colsT(u2, 8).astype(BFNP),
        "inv512": np.full((128, 1), 1.0 / 512.0, dtype=f).astype(BFNP),
        "ivg8": ivg8.astype(F8NP),
        "ones8p": ones8p.astype(F8NP),
        "g2c": colsT(g2.astype(f), CC),
        "identb": np.eye(128, dtype=f).astype(BFNP),
        "epsA": np.full((1, 1), EPS, dtype=f),
        "ones_row": np.ones((1, 512), dtype=f),
        "bq_r": bq.reshape(1, 512).astype(f),
        "br_c": colsT(br_eff, CC),
        "w1bb_r": w1bb.reshape(1, 1024).astype(f),
        "g2b2_r": (g2 * b2).reshape(1, 512).astype(f),
        "be2_c": colsT(be2.astype(f), CC),
    }
    flags = (bool(np.any(bq)), bool(np.any(br_eff)), bool(np.any(w1bb)),
             bool(np.any(b2)), bool(np.any(be2)))
    return consts, b2mean, flags


def kernel(**inputs):
    global LAST_RESULT, B2MEAN_PLACEHOLDER
    import ml_dtypes
    F8NP = ml_dtypes.float8_e4m3
    BFNP = ml_dtypes.bfloat16
    z1 = np.asarray(inputs["z1"], dtype=np.float32)
    z2 = np.asarray(inputs["z2"], dtype=np.float32)
    consts, b2mean, flags = _prep_consts(
        *[np.asarray(inputs[k], dtype=np.float32) for k in
          ["Wq", "bq", "Wk", "bk", "Wv", "bv", "Wr", "br", "g1", "be1",
           "W1", "b1", "W2", "b2", "g2", "be2"]])

    key = ("prog", flags, round(b2mean * 1e9))
    if key not in _CACHE:
        B2MEAN_PLACEHOLDER = b2mean
        _CACHE.clear()
        _CACHE[key] = _build_program(flags)
    nc = _CACHE[key]

    def rearr(a):             # [C, L] -> [128, CC, L]
        return np.ascontiguousarray(a.reshape(CC, 128, L).transpose(1, 0, 2))

    in_maps = []
    for b in range(B):
        m = dict(consts)
        m["z1_8"] = rearr(z1[b]).astype(F8NP)
        m["z2_8"] = rearr(z2[b]).astype(F8NP)
        m["z1res"] = rearr(z1[b]).astype(BFNP)
        in_maps.append(m)

    import os
    trace = bool(int(os.environ.get("KERNEL_TRACE", "0")))
    res = run_bass_kernel_spmd(nc, in_maps, list(range(B)), trace=trace)
    LAST_RESULT = res
    out = np.stack([np.asarray(res.results[b]["out"]).astype(np.float32)
                    for b in range(B)], axis=0)
    return out


B2MEAN_PLACEHOLDER = 0.0
